# revision 15
# baseline (speedup 1.0000x reference)
"""Trainium2 Bass kernel for nn_DenseEntangler (B=256, D=32, L=3, 6 nodes).

Math: out = relu(bias + chain of 6 tensordot contractions). Each per-sample
contraction is a (1024 x 1024) matmul applied to the reshaped state, so the
whole problem is 6 matmuls of [1024,1024]^T @ [1024, Bc*32] per core
(Bc = 32 samples/core on 8 cores, batch-sharded).

Layout scheme (verified against the reference in numpy):
  state XT[(u*32+v) partition, (b*32+f) free], K = 1024 -> 8 tiles of 128.
  steps 0..4:  OUT[(n*32+m), (b,f)] = W_i^T @ XT  with
               W_i[(u*32+v), (n*32+m)] = nodes[i][u,v,m,n]  (host pre-permute)
               transition to the next step's XT = independent aligned 32x32
               block transposes (swap partition-low m with free-low f) ->
               native DVE stream_transpose, runs off the PE critical path.
  step 5:      operands swapped (state stationary, W5 moving) so PSUM comes
               out as [(b*32+f) partition, (m*32+n) free], which is
               DRAM-contiguous per partition for the final store.
Matmuls run as float32r (full PE rate at N>=256); PSUM accumulation is fp32.
"""

import os
import sys

import numpy as np

for _p in ("/opt/trn_rl_repo", "/root/.axon_site/_ro/trn_rl_repo"):
    if _p not in sys.path and os.path.isdir(_p):
        sys.path.append(_p)

B = 256
NCORES = 8
BC = B // NCORES  # 32 samples per core
NSTEP = 6
NK = 8  # K tiles of 128 (K = 1024)
NM = 8  # output partition tiles of 128 (steps 0..4)
NHALF = 2  # halves of 16 samples -> moving free dim 512
HB = BC // NHALF  # 16

_NC_CACHE = {}


def _build_nc(mm_dtype_name):
    import concourse.tile as tile
    from concourse import bacc, mybir

    f32 = mybir.dt.float32
    mmdt = getattr(mybir.dt, mm_dtype_name)

    # Bacc (not plain Bass): its lowering runs move_matmul_waits_to_ldweights
    # + generate_event_semaphores, required to satisfy the HW 1-wait-per-
    # instruction constraint on fused LDWEIGHTS+MATMUL.
    nc = bacc.Bacc(None)
    xh = nc.declare_dram_parameter("x", [BC, 32768], f32, isOutput=False)
    wh = nc.declare_dram_parameter("w", [NSTEP, 128, 8192], f32, isOutput=False)
    bh = nc.declare_dram_parameter("bias_in", [32768], f32, isOutput=False)
    yh = nc.declare_dram_parameter("y", [BC, 32768], f32, isOutput=True)

    # x[b, (k*128+pp)*32 + f] -> [k, pp, b, f]
    x4 = xh[:, :].rearrange("b (k p f) -> k p b f", k=NK, f=32)
    # bias[(f*1024 + q)] -> [f, q]
    b2 = bh[:].rearrange("(f q) -> f q", q=1024)
    # y[b, f*1024 + q] -> [b, f, q]
    y3 = yh[:, :].rearrange("b (f q) -> b f q", q=1024)

    with tile.TileContext(nc) as tc:
        with (
            tc.tile_pool(name="wpool", bufs=16) as wpool,
            tc.tile_pool(name="xpool", bufs=32) as xpool,
            tc.tile_pool(name="bpool", bufs=1) as bpool,
            tc.tile_pool(name="tpool", bufs=4) as tpool,
            tc.tile_pool(name="stpool", bufs=4) as stpool,
            tc.tile_pool(name="opool", bufs=4) as opool,
            tc.tile_pool(name="pspool", bufs=8, space="PSUM") as pspool,
        ):
            wsb = {}

            def load_weights(step):
                # split each step's weight stream across two DGE queues:
                # even k -> gpsimd (SWDGE), odd k -> sync/scalar (HWDGE,
                # alternating by step) so the sustained weight bandwidth
                # (~148 GB/s needed) doesn't sit on a single ~154 GB/s queue.
                hw_eng = nc.sync if step % 2 == 0 else nc.scalar
                tiles = []
                for k in range(NK):
                    t = wpool.tile([128, 1024], mmdt, tag="w")
                    eng = nc.gpsimd if k % 2 == 0 else hw_eng
                    eng.dma_start(
                        out=t[:],
                        in_=wh[step, :, k * 1024 : (k + 1) * 1024].bitcast(mmdt),
                    )
                    tiles.append(t)
                wsb[step] = tiles

            # ---- head: one combined [128, 1024] x tile per k (both halves),
            # alternating the two HWDGE queues; w0 k-tiles interleaved with
            # gpsimd taking the even ones. PE consumes k in arrival order.
            x0 = [None] * NK
            wsb[0] = []
            for k in range(NK):
                tx = xpool.tile([128, BC * 32], mmdt, tag="x0", name=f"x0_{k}", bufs=8)
                # each half on its own HWDGE queue: strided loads run at only
                # ~40 GB/s per queue, so parallelism is what cuts the latency
                nc.sync.dma_start(
                    out=tx[:, 0 : HB * 32].rearrange("p (b f) -> p b f", f=32),
                    in_=x4[k, :, 0:HB, :].bitcast(mmdt),
                )
                nc.scalar.dma_start(
                    out=tx[:, HB * 32 : BC * 32].rearrange("p (b f) -> p b f", f=32),
                    in_=x4[k, :, HB:BC, :].bitcast(mmdt),
                )
                x0[k] = tx
                t = wpool.tile([128, 1024], mmdt, tag="w", name=f"w0_{k}")
                (nc.gpsimd if k % 2 == 0 else (nc.sync if k % 4 == 1 else nc.scalar)).dma_start(
                    out=t[:], in_=wh[0, :, k * 1024 : (k + 1) * 1024].bitcast(mmdt)
                )
                wsb[0].append(t)

            # bias tile (needed only at step 5): [128, 1024], row p holds
            # bias[(p%32)*1024 : ...]
            bias_sb = bpool.tile([128, 1024], f32, tag="bias")
            for r in range(4):
                nc.gpsimd.dma_start(out=bias_sb[32 * r : 32 * (r + 1), :], in_=b2[:, :])

            load_weights(1)

            def finish_tile(ps, h, mt, xt_next):
                """PSUM -> (transpose, round-to-mmdt) -> next-step state tile."""
                if mmdt is f32:
                    t = xpool.tile([128, 512], f32, tag="xt")
                    nc.vector.transpose(t[:], ps[:])
                else:
                    st = stpool.tile([128, 512], f32, tag="st")
                    nc.vector.transpose(st[:], ps[:])
                    t = xpool.tile([128, 512], mmdt, tag="xt")
                    nc.scalar.copy(t[:], st[:])
                xt_next[h][mt] = t

            # ---- step 0, two passes. Pass A (mt 0..3, both halves) runs
            # k-outer so PE consumes k-tiles in DMA arrival order at half the
            # per-k data rate; pass B (mt 4..7) runs from resident data.
            xt_next = [[None] * NK for _ in range(NHALF)]
            pss = [
                [
                    pspool.tile([128, 512], f32, tag="ps", name=f"ps0a_{h}_{i}")
                    for i in range(4)
                ]
                for h in range(NHALF)
            ]
            for k in range(NK):
                for h in range(NHALF):
                    for mt in range(4):
                        nc.tensor.matmul(
                            pss[h][mt][:],
                            wsb[0][k][:, mt * 128 : (mt + 1) * 128],
                            x0[k][:, h * 512 : (h + 1) * 512],
                            start=(k == 0),
                            stop=(k == NK - 1),
                        )
            for h in range(NHALF):
                for mt in range(4):
                    finish_tile(pss[h][mt], h, mt, xt_next)
            for mt in range(4, NM):
                for h in range(NHALF):
                    ps = pspool.tile([128, 512], f32, tag="ps")
                    for k in range(NK):
                        nc.tensor.matmul(
                            ps[:],
                            wsb[0][k][:, mt * 128 : (mt + 1) * 128],
                            x0[k][:, h * 512 : (h + 1) * 512],
                            start=(k == 0),
                            stop=(k == NK - 1),
                        )
                    finish_tile(ps, h, mt, xt_next)
            load_weights(2)
            xt = xt_next

            # ---- steps 1..4: mt-outer (staggers transposes across the step)
            for step in range(1, 5):
                xt_next = [[None] * NK for _ in range(NHALF)]
                for h in range(NHALF):
                    for mt in range(NM):
                        ps = pspool.tile([128, 512], f32, tag="ps")
                        for k in range(NK):
                            nc.tensor.matmul(
                                ps[:],
                                wsb[step][k][:, mt * 128 : (mt + 1) * 128],
                                xt[h][k][:],
                                start=(k == 0),
                                stop=(k == NK - 1),
                            )
                        finish_tile(ps, h, mt, xt_next)
                if step + 2 < NSTEP:
                    load_weights(step + 2)
                xt = xt_next

            # ---- step 5: state stationary, W moving; fused bias+relu+store ----
            from concourse.mybir import ActivationFunctionType

            for h in range(NHALF):
                for mc in range(4):  # output partition chunks of 128 (= 4 b values)
                    for nh in range(2):  # N halves of 512
                        ps = pspool.tile([128, 512], f32, tag="ps")
                        for k in range(NK):
                            nc.tensor.matmul(
                                ps[:],
                                xt[h][k][:, mc * 128 : (mc + 1) * 128],
                                wsb[5][k][:, nh * 512 : (nh + 1) * 512],
                                start=(k == 0),
                                stop=(k == NK - 1),
                            )
                        tmp = tpool.tile([128, 512], f32, tag="tmp")
                        nc.vector.tensor_add(
                            tmp[:], ps[:], bias_sb[:, nh * 512 : (nh + 1) * 512]
                        )
                        o = opool.tile([128, 512], f32, tag="o")
                        nc.scalar.activation(o[:], tmp[:], ActivationFunctionType.Relu)
                        b0 = h * HB + mc * 4
                        nc.sync.dma_start(
                            out=y3[b0 : b0 + 4, :, nh * 512 : (nh + 1) * 512],
                            in_=o[:],
                        )
    # Run the Bacc lowering passes (register allocation, wait splitting, ...)
    # — the PJRT execute path serializes nc.m as-is.
    nc.finalize()
    return nc


def _get_nc(mm_dtype_name):
    if mm_dtype_name not in _NC_CACHE:
        _NC_CACHE[mm_dtype_name] = _build_nc(mm_dtype_name)
    return _NC_CACHE[mm_dtype_name]


def _prep_weights(nodes):
    # W[i] layout [p=(u*32+v)%... rows 128 per k-tile packed as [128, 8*1024]]:
    # free index = k*1024 + col.  steps 0..4: col = n*32+m ; step 5: col = m*32+n.
    nodes = np.ascontiguousarray(nodes, dtype=np.float32)
    W = np.empty((NSTEP, 128, 8192), np.float32)
    for i in range(NSTEP):
        if i < 5:
            wm = nodes[i].reshape(1024, 32, 32).transpose(0, 2, 1).reshape(1024, 1024)
        else:
            wm = nodes[i].reshape(1024, 1024)
        # [k*128+p, col] -> [p, k*1024+col]
        W[i] = wm.reshape(NK, 128, 1024).transpose(1, 0, 2).reshape(128, 8192)
    return W


def run(inputs, nodes, bias, mm_dtype="float32r", trace=False):
    from concourse.bass_utils import run_bass_kernel_spmd

    nc = _get_nc(mm_dtype)
    x = np.ascontiguousarray(inputs, dtype=np.float32)
    bias = np.ascontiguousarray(bias, dtype=np.float32)
    W = _prep_weights(nodes)
    in_maps = [
        {"x": x[c * BC : (c + 1) * BC], "w": W, "bias_in": bias}
        for c in range(NCORES)
    ]
    res = run_bass_kernel_spmd(nc, in_maps, list(range(NCORES)), trace=trace)
    out = np.concatenate([res.results[c]["y"] for c in range(NCORES)], axis=0)
    return out, res


def kernel(inputs, nodes, bias):
    out, _ = run(inputs, nodes, bias)
    return out


# revision 16
# speedup vs baseline: 1.0139x; 1.0139x over previous
"""Trainium2 Bass kernel for nn_DenseEntangler (B=256, D=32, L=3, 6 nodes).

Math: out = relu(bias + chain of 6 tensordot contractions). Each per-sample
contraction is a (1024 x 1024) matmul applied to the reshaped state, so the
whole problem is 6 matmuls of [1024,1024]^T @ [1024, Bc*32] per core
(Bc = 32 samples/core on 8 cores, batch-sharded).

Layout scheme (verified against the reference in numpy):
  state XT[(u*32+v) partition, (b*32+f) free], K = 1024 -> 8 tiles of 128.
  steps 0..4:  OUT[(n*32+m), (b,f)] = W_i^T @ XT  with
               W_i[(u*32+v), (n*32+m)] = nodes[i][u,v,m,n]  (host pre-permute)
               transition to the next step's XT = independent aligned 32x32
               block transposes (swap partition-low m with free-low f) ->
               native DVE stream_transpose, runs off the PE critical path.
  step 5:      operands swapped (state stationary, W5 moving) so PSUM comes
               out as [(b*32+f) partition, (m*32+n) free], which is
               DRAM-contiguous per partition for the final store.
Matmuls run as float32r (full PE rate at N>=256); PSUM accumulation is fp32.
"""

import os
import sys

import numpy as np

for _p in ("/opt/trn_rl_repo", "/root/.axon_site/_ro/trn_rl_repo"):
    if _p not in sys.path and os.path.isdir(_p):
        sys.path.append(_p)

B = 256
NCORES = 8
BC = B // NCORES  # 32 samples per core
NSTEP = 6
NK = 8  # K tiles of 128 (K = 1024)
NM = 8  # output partition tiles of 128 (steps 0..4)
NHALF = 2  # halves of 16 samples -> moving free dim 512
HB = BC // NHALF  # 16

_NC_CACHE = {}


def _build_nc(mm_dtype_name):
    import concourse.tile as tile
    from concourse import bacc, mybir

    f32 = mybir.dt.float32
    mmdt = getattr(mybir.dt, mm_dtype_name)

    # Bacc (not plain Bass): its lowering runs move_matmul_waits_to_ldweights
    # + generate_event_semaphores, required to satisfy the HW 1-wait-per-
    # instruction constraint on fused LDWEIGHTS+MATMUL.
    nc = bacc.Bacc(None)
    xh = nc.declare_dram_parameter("x", [BC, 32768], f32, isOutput=False)
    wh = nc.declare_dram_parameter("w", [NSTEP, 128, 8192], f32, isOutput=False)
    bh = nc.declare_dram_parameter("bias_in", [32768], f32, isOutput=False)
    yh = nc.declare_dram_parameter("y", [BC, 32768], f32, isOutput=True)

    # x[b, (k*128+pp)*32 + f] -> [k, pp, b, f]
    x4 = xh[:, :].rearrange("b (k p f) -> k p b f", k=NK, f=32)
    # bias[(f*1024 + q)] -> [f, q]
    b2 = bh[:].rearrange("(f q) -> f q", q=1024)
    # y[b, f*1024 + q] -> [b, f, q]
    y3 = yh[:, :].rearrange("b (f q) -> b f q", q=1024)

    with tile.TileContext(nc) as tc:
        with (
            tc.tile_pool(name="wpool", bufs=16) as wpool,
            tc.tile_pool(name="xpool", bufs=32) as xpool,
            tc.tile_pool(name="bpool", bufs=1) as bpool,
            tc.tile_pool(name="tpool", bufs=4) as tpool,
            tc.tile_pool(name="stpool", bufs=4) as stpool,
            tc.tile_pool(name="opool", bufs=4) as opool,
            tc.tile_pool(name="pspool", bufs=8, space="PSUM") as pspool,
        ):
            wsb = {}

            def load_weights(step):
                # split each step's weight stream across two DGE queues:
                # even k -> gpsimd (SWDGE), odd k -> sync/scalar (HWDGE,
                # alternating by step) so the sustained weight bandwidth
                # (~148 GB/s needed) doesn't sit on a single ~154 GB/s queue.
                hw_eng = nc.sync if step % 2 == 0 else nc.scalar
                tiles = []
                for k in range(NK):
                    t = wpool.tile([128, 1024], mmdt, tag="w")
                    eng = nc.gpsimd if k % 2 == 0 else hw_eng
                    eng.dma_start(
                        out=t[:],
                        in_=wh[step, :, k * 1024 : (k + 1) * 1024].bitcast(mmdt),
                    )
                    tiles.append(t)
                wsb[step] = tiles

            # ---- head: one combined [128, 1024] x tile per k (both halves),
            # alternating the two HWDGE queues; w0 k-tiles interleaved with
            # gpsimd taking the even ones. PE consumes k in arrival order.
            x0 = [None] * NK
            wsb[0] = []
            for k in range(NK):
                tx = xpool.tile([128, BC * 32], mmdt, tag="x0", name=f"x0_{k}", bufs=8)
                # strided loads run at only ~40 GB/s per queue; alternate the
                # two HWDGE queues per (k, half) so each half-stream gets 2x.
                qa, qb = (nc.sync, nc.scalar) if k % 2 == 0 else (nc.scalar, nc.sync)
                qa.dma_start(
                    out=tx[:, 0 : HB * 32].rearrange("p (b f) -> p b f", f=32),
                    in_=x4[k, :, 0:HB, :].bitcast(mmdt),
                )
                qb.dma_start(
                    out=tx[:, HB * 32 : BC * 32].rearrange("p (b f) -> p b f", f=32),
                    in_=x4[k, :, HB:BC, :].bitcast(mmdt),
                )
                x0[k] = tx
                t = wpool.tile([128, 1024], mmdt, tag="w", name=f"w0_{k}")
                nc.gpsimd.dma_start(
                    out=t[:], in_=wh[0, :, k * 1024 : (k + 1) * 1024].bitcast(mmdt)
                )
                wsb[0].append(t)

            load_weights(1)

            def finish_tile(ps, h, mt, xt_next):
                """PSUM -> (transpose, round-to-mmdt) -> next-step state tile."""
                if mmdt is f32:
                    t = xpool.tile([128, 512], f32, tag="xt")
                    nc.vector.transpose(t[:], ps[:])
                else:
                    st = stpool.tile([128, 512], f32, tag="st")
                    nc.vector.transpose(st[:], ps[:])
                    t = xpool.tile([128, 512], mmdt, tag="xt")
                    nc.scalar.copy(t[:], st[:])
                xt_next[h][mt] = t

            # ---- step 0: k-outer so PE consumes k-tiles in DMA arrival order
            xt_next = [[None] * NK for _ in range(NHALF)]
            for h in range(NHALF):
                pss = [
                    pspool.tile([128, 512], f32, tag="ps", name=f"ps0_{h}_{i}")
                    for i in range(NM)
                ]
                for k in range(NK):
                    for mt in range(NM):
                        nc.tensor.matmul(
                            pss[mt][:],
                            wsb[0][k][:, mt * 128 : (mt + 1) * 128],
                            x0[k][:, h * 512 : (h + 1) * 512],
                            start=(k == 0),
                            stop=(k == NK - 1),
                        )
                for mt in range(NM):
                    finish_tile(pss[mt], h, mt, xt_next)
            load_weights(2)
            xt = xt_next

            # ---- steps 1..4: mt-outer (staggers transposes across the step)
            for step in range(1, 5):
                xt_next = [[None] * NK for _ in range(NHALF)]
                for h in range(NHALF):
                    for mt in range(NM):
                        ps = pspool.tile([128, 512], f32, tag="ps")
                        for k in range(NK):
                            nc.tensor.matmul(
                                ps[:],
                                wsb[step][k][:, mt * 128 : (mt + 1) * 128],
                                xt[h][k][:],
                                start=(k == 0),
                                stop=(k == NK - 1),
                            )
                        finish_tile(ps, h, mt, xt_next)
                if step + 2 < NSTEP:
                    load_weights(step + 2)
                xt = xt_next

            # ---- step 5: state stationary, W moving; fused bias+relu+store ----
            from concourse.mybir import ActivationFunctionType

            # bias tile: [128, 1024], row p holds bias[(p%32)*1024 : ...];
            # loaded late, right before its only consumer.
            bias_sb = bpool.tile([128, 1024], f32, tag="bias")
            for r in range(4):
                nc.sync.dma_start(out=bias_sb[32 * r : 32 * (r + 1), :], in_=b2[:, :])

            for h in range(NHALF):
                for mc in range(4):  # output partition chunks of 128 (= 4 b values)
                    for nh in range(2):  # N halves of 512
                        ps = pspool.tile([128, 512], f32, tag="ps")
                        for k in range(NK):
                            nc.tensor.matmul(
                                ps[:],
                                xt[h][k][:, mc * 128 : (mc + 1) * 128],
                                wsb[5][k][:, nh * 512 : (nh + 1) * 512],
                                start=(k == 0),
                                stop=(k == NK - 1),
                            )
                        tmp = tpool.tile([128, 512], f32, tag="tmp")
                        nc.vector.tensor_add(
                            tmp[:], ps[:], bias_sb[:, nh * 512 : (nh + 1) * 512]
                        )
                        o = opool.tile([128, 512], f32, tag="o")
                        nc.scalar.activation(o[:], tmp[:], ActivationFunctionType.Relu)
                        b0 = h * HB + mc * 4
                        nc.sync.dma_start(
                            out=y3[b0 : b0 + 4, :, nh * 512 : (nh + 1) * 512],
                            in_=o[:],
                        )
    # Run the Bacc lowering passes (register allocation, wait splitting, ...)
    # — the PJRT execute path serializes nc.m as-is.
    nc.finalize()
    return nc


def _get_nc(mm_dtype_name):
    if mm_dtype_name not in _NC_CACHE:
        _NC_CACHE[mm_dtype_name] = _build_nc(mm_dtype_name)
    return _NC_CACHE[mm_dtype_name]


def _prep_weights(nodes):
    # W[i] layout [p=(u*32+v)%... rows 128 per k-tile packed as [128, 8*1024]]:
    # free index = k*1024 + col.  steps 0..4: col = n*32+m ; step 5: col = m*32+n.
    nodes = np.ascontiguousarray(nodes, dtype=np.float32)
    W = np.empty((NSTEP, 128, 8192), np.float32)
    for i in range(NSTEP):
        if i < 5:
            wm = nodes[i].reshape(1024, 32, 32).transpose(0, 2, 1).reshape(1024, 1024)
        else:
            wm = nodes[i].reshape(1024, 1024)
        # [k*128+p, col] -> [p, k*1024+col]
        W[i] = wm.reshape(NK, 128, 1024).transpose(1, 0, 2).reshape(128, 8192)
    return W


def run(inputs, nodes, bias, mm_dtype="float32r", trace=False):
    from concourse.bass_utils import run_bass_kernel_spmd

    nc = _get_nc(mm_dtype)
    x = np.ascontiguousarray(inputs, dtype=np.float32)
    bias = np.ascontiguousarray(bias, dtype=np.float32)
    W = _prep_weights(nodes)
    in_maps = [
        {"x": x[c * BC : (c + 1) * BC], "w": W, "bias_in": bias}
        for c in range(NCORES)
    ]
    res = run_bass_kernel_spmd(nc, in_maps, list(range(NCORES)), trace=trace)
    out = np.concatenate([res.results[c]["y"] for c in range(NCORES)], axis=0)
    return out, res


def kernel(inputs, nodes, bias):
    out, _ = run(inputs, nodes, bias)
    return out
